# revision 57
# baseline (speedup 1.0000x reference)
"""Trainium2 Bass kernel for nn_MultiHeadAttention_48395691492077.

Reference (B=4, S=2048, D=1024, single head, anti-causal triu mask):
    qkv = x @ wqkv; q,k,v = split(qkv)
    scores = triu(q @ k^T / sqrt(B));  masked softmax over keys t >= s
    x2  = softmax(scores) @ v @ w_lin + b_lin + x
    out = relu(x2 @ w_ff1 + b_ff1) @ w_ff2 + b_ff2 + x2

Sharding: 8 cores = 4 batches x 2 query-halves. Each core computes
attention + MLP for its own 1024 queries against the full 2048-key
sequence of its batch. The program is identical on all cores (SPMD);
per-core differences (which queries, which mask pattern) are carried in
the input data plus one branch on the query-parity register.

Device algebra (transposed; no on-chip transposes, no K/V projections):
    uT = wzq^T.T @ qxT            with wzq = (Wq @ Wk^T)/2  (host-fused)
    scoresT[t,s] = sum_d xT[d,t] * uT[d,s]     (keys are raw x!)
    expT = exp(scoresT); diagonal 128-strips *= lower-tri mask
    den[s] broadcast = ones[128,128].T @ expT (PE), rbs = 1/den (DVE)
    H^T[d,s] = x[t,d].T @ expT  (A@X; V projection deferred)
    attnT = H^T * rbs
    x2T = wvl.T @ attnT + (xT + b_lin)  with wvl = Wv @ w_lin (host-fused:
          A@(X@Wv)@w_lin == (A@X)@(Wv@w_lin) by associativity)
    hT = relu(w_ff1.T @ x2T + b_ff1)
    outT = w_ff2.T @ hT + x2T               (+ b_ff2 added on host)
Blocks crossing the anti-causal diagonal use exact matmul widths
(128/256/384) instead of full 512; PSUM accumulation runs widest-first
so every column is initialized by the start=True matmul.
Matmul inputs are bf16 (fp32 PSUM accumulation); residuals are fp32.
"""

import numpy as np
import ml_dtypes

B, S, D = 4, 2048, 1024
NCORES = 8
BF16 = ml_dtypes.bfloat16

NT = S // 128            # 16 t-chunks
ND = D // 128            # 8 chunks of 128 along any D-sized dim

# global query-column starts of (sb0, sb1) per parity
SB_GLOBAL = {0: (0, 1536), 1: (512, 1024)}
# t-chunks each (parity, s-block) actually needs (branch-specialized)
SB_SLOTS = {
    p: {sb: list(range(SB_GLOBAL[p][sb] // 128, NT)) for sb in (0, 1)}
    for p in (0, 1)
}


def _width(parity, sb, tc):
    """Valid column count of block (sb, tc): cols [0, w) of the 512."""
    return min(512, 128 * tc - SB_GLOBAL[parity][sb] + 128)


def _is_diag(parity, sb, tc):
    """Block whose last 128 columns lie on the anti-causal diagonal."""
    return tc - SB_GLOBAL[parity][sb] // 128 < 4


_COMPILED = None
_LAST_IN_MAPS = None


def _build_program():
    from contextlib import ExitStack
    import concourse.bacc as bacc
    import concourse.mybir as mybir
    import concourse.tile as tile

    f32 = mybir.dt.float32
    b16 = mybir.dt.bfloat16
    AF = mybir.ActivationFunctionType

    nc = bacc.Bacc("TRN2", target_bir_lowering=False, debug=False,
                   num_devices=NCORES)

    # all big inputs arrive pre-arranged on the host into the on-chip
    # [128, chunk, free] layout so every DMA is contiguous per partition
    xT_d = nc.dram_tensor("xT", [128, ND * S], b16, kind="ExternalInput")
    xn_d = nc.dram_tensor("xn", [128, NT * D], b16, kind="ExternalInput")
    qxT_d = nc.dram_tensor("qxT", [128, ND * 1024], b16, kind="ExternalInput")
    xq_d = nc.dram_tensor("xq", [D, 1024], f32, kind="ExternalInput")
    wzq_d = nc.dram_tensor("wzq", [128, ND * D], b16, kind="ExternalInput")
    wvl_d = nc.dram_tensor("wvl", [128, ND * D], b16, kind="ExternalInput")
    f8 = mybir.dt.float8e4
    wff1_d = nc.dram_tensor("wff1", [128, ND * D], f8, kind="ExternalInput")
    wff2_d = nc.dram_tensor("wff2", [128, ND * D], f8, kind="ExternalInput")
    tri_d = nc.dram_tensor("tri", [128, 128], b16, kind="ExternalInput")
    par_d = nc.dram_tensor("par", [1, 1], mybir.dt.uint32, kind="ExternalInput")
    bf1_d = nc.dram_tensor("bf1", [ND, 128], f32, kind="ExternalInput")
    outT_d = nc.dram_tensor("outT", [D, 1024], f32, kind="ExternalOutput")

    with tile.TileContext(nc) as tc:
        es = ExitStack()
        with es:
            pp = es.enter_context(tc.tile_pool(name="persist", bufs=1))
            sp = es.enter_context(tc.tile_pool(name="stream", bufs=2))
            ps = es.enter_context(
                tc.tile_pool(name="ps", bufs=8, space="PSUM"))
            esB = es.enter_context(ExitStack())
            pb = esB.enter_context(tc.tile_pool(name="pB", bufs=1))
            pr = es.enter_context(tc.tile_pool(name="pAC", bufs=1,
                                               side="right"))

            def psum():
                t = ps.tile([128, 512], f32, tag="mm", bufs=8, name="mmps")
                return t

            # ---- constants ----
            # load the parity register up front so every engine sequencer
            # resolves it during the startup DMA wait, not at the branch
            par_regs = nc.alloc_registers("par_regs")
            nc.regs_load(par_regs, par_d.ap()[0:1, 0:1])
            par = nc.snap(par_regs, donate=True, min_val=0, max_val=1)

            ones_sq = pp.tile([128, 128], b16, tag="ones_sq", bufs=1)
            nc.vector.memset(ones_sq[:], 1.0)
            tri_t = pp.tile([128, 128], b16, tag="tri", bufs=1)
            # warm the PE HAM clock-gate while the first input DMAs land
            wups = psum()
            for i in range(16):
                nc.tensor.matmul(wups[:, 0:128], ones_sq[:], ones_sq[:],
                                 start=(i == 0), stop=(i == 15))

            # ---- input loads (arrival-ordered for phase-A pipelining).
            # Descriptor generation serializes per issuing queue (~0.7us per
            # dma_start), so the early loads fan out across engine queues.
            def chunks(dram, c0, c1, width):
                return dram.ap()[:, c0 * width:c1 * width].rearrange(
                    "p (c n) -> p c n", n=width)

            wzq_a = pr.tile([128, ND, D], b16, tag="wzq", bufs=1)
            qx_a = pr.tile([128, ND, 1024], b16, tag="qx", bufs=1)
            # single-a granules so the a-outer phase-A loop starts as soon
            # as wzq[a0] + the sb0 half of qx[a0] land
            for a in range(ND):
                nc.sync.dma_start(wzq_a[:, a:a + 1], chunks(wzq_d, a, a + 1, D))
                nc.sync.dma_start(
                    qx_a[:, a:a + 1, 0:512],
                    qxT_d.ap()[:, a * 1024:a * 1024 + 512]
                    .rearrange("p (c n) -> p c n", n=512))
            nc.sync.dma_start(
                qx_a[:, :, 512:1024],
                qxT_d.ap().rearrange("p (c n) -> p c n", n=1024)[:, :, 512:1024])
            # xT feeds the scores pass; host interleaves it so each 512-col
            # chunk is flat-contiguous (full-bandwidth descriptors). Chunk
            # order serves both parities' first pass-1 blocks, then the
            # descending tail.
            xt_a = pb.tile([128, 4, ND, 512], b16, tag="xt", bufs=1)
            for cc in (1, 0, 3, 2):
                nc.sync.dma_start(
                    xt_a[:, cc],
                    xT_d.ap()[:, cc * 4096:(cc + 1) * 4096]
                    .rearrange("p (a n) -> p a n", n=512))
            # x natural layout [t, d] feeds the A@X pass (later still)
            xn_a = pb.tile([128, NT, D], b16, tag="xn", bufs=1)
            nc.sync.dma_start(xn_a[:], chunks(xn_d, 0, NT, D))
            nc.sync.dma_start(tri_t[:], tri_d.ap())
            # b_ff1 laid out [128, ND]: bias column fc serves f-chunk fc
            bf1_t = pp.tile([128, ND], f32, tag="bf1", bufs=1)
            nc.sync.dma_start(bf1_t[:], bf1_d.ap().rearrange("c p -> p c"))
            wzq_t = [wzq_a[:, d] for d in range(ND)]
            xn = [xn_a[:, t] for t in range(NT)]
            qx = [qx_a[:, d] for d in range(ND)]

            def xts(d, tcn):
                j = tcn % 4
                return xt_a[:, tcn // 4, d, j * 128:(j + 1) * 128]

            # ---- phase A: uT[d, s] = sum_a wzq[a,d] * qxT[a,s] ----
            # a-outer in two sb-halves (8 PSUM banks each, all m per half):
            # compute starts once wzq[a0]+qx[a0,sb0] land, and the sb0 ut
            # evictions (which gate the first scores blocks) overlap the
            # whole sb1 half.
            ut = [pb.tile([128, 1024], b16, tag=f"ut{m}", bufs=1,
                          name=f"ut{m}") for m in range(ND)]

            def phase_a(sb, a_range, ups):
                for a in a_range:
                    for m in range(ND):
                        nc.tensor.matmul(
                            ups[m][:],
                            wzq_t[a][:, m * 128:(m + 1) * 128],
                            qx[a][:, sb * 512:(sb + 1) * 512],
                            start=(a == 0), stop=(a == ND - 1))

            def evict_u(sb, ups):
                # alternate vector / scalar so the eviction chain halves
                for m in range(ND):
                    dst = ut[m][:, sb * 512:(sb + 1) * 512]
                    if m % 2 == 0:
                        nc.vector.tensor_copy(dst, ups[m][:])
                    else:
                        nc.scalar.activation(dst, ups[m][:], AF.Copy)

            def phase_a_all():
                for sb in range(2):
                    ups = {m: psum() for m in range(ND)}
                    phase_a(sb, range(ND), ups)
                    evict_u(sb, ups)

            # phase-C weights prefetch into the same right pool (wzq/qx stay
            # live through the in-branch phase-A tail; fp8 weights fit all)
            wl_a = pr.tile([128, ND, D], b16, tag="wl", bufs=1)
            nc.sync.dma_start(wl_a[:], chunks(wvl_d, 0, ND, D))
            wf1_a = pr.tile([128, ND, D], f8, tag="wf1", bufs=1)
            nc.sync.dma_start(wf1_a[:], chunks(wff1_d, 0, ND, D))
            wf2_a = pr.tile([128, ND, D], f8, tag="wf2", bufs=1)
            nc.sync.dma_start(wf2_a[:], chunks(wff2_d, 0, ND, D))
            wvl_t = [wl_a[:, d] for d in range(ND)]

            attn = [pr.tile([128, 1024], b16, tag=f"at{d}", bufs=1,
                            name=f"at{d}") for d in range(ND)]

            def phase_b(parity):
                sb_slots = SB_SLOTS[parity]
                # pass 1: scoresT -> exp -> diag mask, tc-outer.
                # tc_min first (it only needs the sb0 half of ut), then
                # descending so den's widest-first operands are ready early
                # and the last exp (narrow) barely gates pass 2.
                et = {}
                tc_min = sb_slots[0][0]
                order = [tc_min, tc_min + 1, tc_min + 2] + \
                    list(range(NT - 1, tc_min + 2, -1))
                for tcn in order:
                    work = [(sb, _width(parity, sb, tcn))
                            for sb in (0, 1) if tcn in sb_slots[sb]]
                    scp = {sb: psum() for sb, _ in work}
                    for d in range(ND):
                        for sb, w in work:
                            nc.tensor.matmul(
                                scp[sb][:, 0:w],
                                xts(d, tcn),
                                ut[d][:, sb * 512:sb * 512 + w],
                                start=(d == 0), stop=(d == ND - 1))
                    for sb, w in work:
                        e = pb.tile([128, w], b16, tag=f"et{sb}_{tcn}",
                                    bufs=1, name=f"et{parity}_{sb}_{tcn}")
                        et[(sb, tcn)] = e
                        nc.scalar.activation(e[:], scp[sb][:, 0:w], AF.Exp)
                        if _is_diag(parity, sb, tcn):
                            nc.vector.tensor_mul(
                                e[:, w - 128:w], e[:, w - 128:w], tri_t[:])

                # pass 2: den (broadcast), recip, A@X, normalize.
                # Accumulate widest-first (descending tc) so the start=True
                # matmul initializes the full 512 columns.
                rbs = {}
                for sb in (0, 1):
                    slots = sb_slots[sb][::-1]
                    den_ps = psum()
                    for k, tcn in enumerate(slots):
                        w = _width(parity, sb, tcn)
                        nc.tensor.matmul(
                            den_ps[:, 0:w], ones_sq[:], et[(sb, tcn)][:],
                            start=(k == 0), stop=(k == len(slots) - 1))
                    r = sp.tile([128, 512], f32, tag="rbs", bufs=2,
                                name=f"rbs{parity}_{sb}")
                    nc.vector.reciprocal(r[:], den_ps[:])
                    rbs[sb] = r

                for dc in range(ND):
                    axp = {sb: psum() for sb in (0, 1)}
                    for sb in (0, 1):
                        slots = sb_slots[sb][::-1]
                        for k, tcn in enumerate(slots):
                            w = _width(parity, sb, tcn)
                            nc.tensor.matmul(
                                axp[sb][:, 0:w],
                                xn[tcn][:, dc * 128:(dc + 1) * 128],
                                et[(sb, tcn)][:],
                                start=(k == 0),
                                stop=(k == len(slots) - 1))
                    for sb in (0, 1):
                        nc.vector.tensor_mul(
                            attn[dc][:, sb * 512:(sb + 1) * 512],
                            axp[sb][:], rbs[sb][:])

            # the entire phase A + B sits inside both branch bodies; the
            # branch is resolved right after warmup dispatch, overlapping
            # the startup DMA wait instead of stalling the PE mid-kernel
            with tc.If(par < 1) as cmp:
                phase_a_all()
                phase_b(0)
            with cmp.Else():
                phase_a_all()
                phase_b(1)

            # ---- free pB (ut/xt/xn/et); left pool for phase-C tiles ----
            esB.close()
            esC = es.enter_context(ExitStack())
            pc = esC.enter_context(tc.tile_pool(name="pC", bufs=1))

            x2f = [pc.tile([128, 1024], f32, tag=f"x2f{d}", bufs=1,
                           name=f"x2f{d}") for d in range(ND)]
            # Both FFN GEMMs run in fp8 DoubleRow. Scale chain: x2f carries
            # 32x (host scaled wvl/xq by 32); x2b = x2f/32 is true x2 in fp8;
            # w_ff1/w_ff2 are host-scaled by 32 into fp8's normal range; the
            # relu's scale=1/32 keeps ht exact; the final 32x output factor
            # is divided out on the host.
            x2b_a = pc.tile([128, ND, 1024], f8, tag="x2b", bufs=1)
            ht_a = pc.tile([128, ND, 1024], f8, tag="ht", bufs=1)

            # s2-halved pipeline: ff2(0)'s output adds (DVE) overlap ff1(1)'s
            # matmuls, so only ff2(1)'s tail is exposed past the last matmul
            def wvl_half(s2):
                cc = slice(s2 * 512, (s2 + 1) * 512)
                for oc in range(ND):
                    cps = psum()
                    for d in range(ND):
                        nc.tensor.matmul(
                            cps[:],
                            wvl_t[d][:, oc * 128:(oc + 1) * 128],
                            attn[d][:, cc],
                            start=(d == 0), stop=(d == ND - 1))
                    xqt = sp.tile([128, 512], f32, tag="xqt", bufs=4,
                                  name=f"xqt{oc}_{s2}")
                    nc.sync.dma_start(
                        xqt[:],
                        xq_d.ap()[oc * 128:(oc + 1) * 128, cc])
                    nc.vector.tensor_add(x2f[oc][:, cc], cps[:], xqt[:])
                    nc.vector.tensor_scalar_mul(x2b_a[:, oc, cc],
                                                x2f[oc][:, cc], 1.0 / 32)

            def ff1_half(s2):
                cc = slice(s2 * 512, (s2 + 1) * 512)
                for fc in range(ND):
                    cps = psum()
                    for d2 in range(0, ND, 2):
                        nc.tensor.matmul(
                            cps[:],
                            wf1_a[:, d2:d2 + 2, fc * 128:(fc + 1) * 128],
                            x2b_a[:, d2:d2 + 2, cc],
                            start=(d2 == 0), stop=(d2 == ND - 2),
                            perf_mode=mybir.MatmulPerfMode.DoubleRow)
                    nc.scalar.activation(ht_a[:, fc, cc], cps[:], AF.Relu,
                                         bias=bf1_t[:, fc:fc + 1],
                                         scale=1.0 / 32)

            def ff2_half(s2):
                cc = slice(s2 * 512, (s2 + 1) * 512)
                for oc in range(ND):
                    cps = psum()
                    for f2 in range(0, ND, 2):
                        nc.tensor.matmul(
                            cps[:],
                            wf2_a[:, f2:f2 + 2, oc * 128:(oc + 1) * 128],
                            ht_a[:, f2:f2 + 2, cc],
                            start=(f2 == 0), stop=(f2 == ND - 2),
                            perf_mode=mybir.MatmulPerfMode.DoubleRow)
                    ot = sp.tile([128, 512], f32, tag="ot", bufs=4,
                                 name=f"ot{oc}_{s2}")
                    nc.vector.tensor_add(ot[:], cps[:], x2f[oc][:, cc])
                    nc.gpsimd.dma_start(
                        outT_d.ap()[oc * 128:(oc + 1) * 128, cc], ot[:])

            wvl_half(0)
            wvl_half(1)
            ff1_half(0)
            ff2_half(0)
            ff1_half(1)
            ff2_half(1)

    nc.compile()
    return nc


def _get_program():
    global _COMPILED
    if _COMPILED is None:
        _COMPILED = _build_program()
    return _COMPILED


def _p128(arr):
    """[c*128, C] -> [128, c*C]: the on-chip chunked layout, so device DMAs
    are contiguous per partition."""
    c = arr.shape[0] // 128
    return np.ascontiguousarray(
        arr.reshape(c, 128, -1).transpose(1, 0, 2).reshape(128, -1))


def _p128_xt(xT):
    """xT [D, S] -> [128, (cc, a, 512)]: 512-col chunks flat-contiguous so
    each chunk loads with full-bandwidth descriptors."""
    a = xT.reshape(ND, 128, 4, 512)            # [a, p, cc, n]
    return np.ascontiguousarray(
        a.transpose(1, 2, 0, 3).reshape(128, -1))


def kernel(x, wqkv, w_lin, b_lin, w_ff1, b_ff1, w_ff2, b_ff2):
    from concourse.bass_utils import run_bass_kernel_spmd

    x = np.asarray(x, np.float32)
    wqkv = np.asarray(wqkv, np.float32)
    Wq = wqkv[:, :D].astype(np.float64)
    Wk = wqkv[:, D:2 * D].astype(np.float64)
    Wv = wqkv[:, 2 * D:].astype(np.float64)

    F8 = ml_dtypes.float8_e4m3
    wzq = _p128(((Wq @ Wk.T) / 2.0).astype(BF16))   # [a, d] natural layout
    # phase C carries a 32x scale (wvl, xq) so x2b = x2f/32 is exact x2;
    # both FFN weights are scaled by 32 into fp8's normal range; the final
    # 32x on the output is divided out below
    wvl = _p128((Wv @ np.asarray(w_lin, np.float64) * 32.0).astype(BF16))
    wff1 = _p128((np.asarray(w_ff1, np.float32) * 32.0).astype(F8))
    wff2 = _p128((np.asarray(w_ff2, np.float32) * 32.0).astype(F8))
    tri = (np.arange(128)[:, None] >= np.arange(128)[None, :]).astype(BF16)

    in_maps = []
    qcols_by_parity = {
        0: np.r_[0:512, 1536:2048],
        1: np.r_[512:1536],
    }
    b_lin = np.asarray(b_lin, np.float32)
    b_ff1 = np.asarray(b_ff1, np.float32)
    b_ff2 = np.asarray(b_ff2, np.float32)
    bf1 = np.ascontiguousarray(b_ff1.reshape(ND, 128))
    for c in range(NCORES):
        b, h = c // 2, c % 2
        xT32 = np.ascontiguousarray(x[b].T)               # [D, S] f32
        qcols = qcols_by_parity[h]
        qxT32 = np.ascontiguousarray(xT32[:, qcols])      # [D, 1024]
        in_maps.append({
            "xT": _p128_xt(xT32.astype(BF16)),
            "xn": _p128(x[b].astype(BF16)),               # [S, D] natural
            "qxT": _p128(qxT32.astype(BF16)),
            "xq": (qxT32 + b_lin[:, None]) * 32.0,        # b_lin folded in
            "wzq": wzq,
            "wvl": wvl,
            "wff1": wff1,
            "wff2": wff2,
            "tri": tri,
            "bf1": bf1,
            "par": np.full((1, 1), h, np.uint32),
        })

    global _LAST_IN_MAPS
    _LAST_IN_MAPS = in_maps
    nc = _get_program()
    res = run_bass_kernel_spmd(nc, in_maps, core_ids=list(range(NCORES)))

    out = np.empty((B, S, D), np.float32)
    for c in range(NCORES):
        b, h = c // 2, c % 2
        ol = res.results[c]["outT"].T / 32.0              # [1024 s, D]
        if h == 0:
            out[b, 0:512] = ol[:512]
            out[b, 1536:2048] = ol[512:]
        else:
            out[b, 512:1536] = ol
    out += b_ff2[None, None, :]
    return out
